# revision 2
# baseline (speedup 1.0000x reference)
"""PointGRN (segment_reduce) Trainium2 Bass kernel, v2.

Computation (per segment b, channel c over points feat [N, 64] f32):
    sumsq[b,c]  = sum_{n in seg b} feat[n,c]^2
    r[b,c]      = sqrt(sumsq[b,c])
    rn[b,c]     = r[b,c] / (mean_c r[b,:] + 1e-6)
    out[n,c]    = feat[n,c] * (1 + gamma[c]*rn[b,c]) + beta[c]

Sharding: data-parallel over segments — host reads `offset` and gives each
of the 8 cores one whole segment (padded with zero rows to a 128-row
multiple).  No device-side searchsorted and no collectives needed.

v2 changes over the 121.8us baseline (which was pass-2 DVE-bound: two fp16
tensor_tensor ops at 2x packed mode = 246 G elem/s = ~65us):
  * HBM output tensor is fp16, upcast to f32 on host.  The staged w tiles
    were already fp16, so the stored values are bit-identical to what the
    old casting-DMA/upcast path produced — zero added error.  Stores are
    plain fp16->fp16 DMAs on the sync/scalar HWDGE rings (16 MB instead of
    32 MB of HBM writes), which frees GPSIMD from all store duty.
  * Pass-2 tiles are split between DVE and GPSIMD (gp_tiles of nt tiles run
    both tensor_tensor ops on GPSIMD at ~half DVE rate) so the two engines
    finish together: ~44us instead of 65.
Pass 1 is unchanged (DMA-bound ~61us): SWDGE f32 loads, DVE convert to the
fp16 resident copy, ACT squares into bf16, PE ones-matmul partition-reduce
into a 4-bank PSUM row; short serial combine chain.
"""

import numpy as np

import concourse.bacc as bacc
import concourse.bass as bass
import concourse.mybir as mybir
import concourse.tile as tile
from concourse.bass_utils import run_bass_kernel_spmd

EPS = 1e-06
N_CORES = 8
P = 128          # SBUF partitions
C = 64           # channels
K = 32           # row-groups per partition per full tile
F = K * C        # full-tile free dim (2048 f32 = 8KB/partition)
MM_N = 512       # matmul moving free-dim chunk (one PSUM bank)

_AFT = mybir.ActivationFunctionType
_ALU = mybir.AluOpType

_program_cache: dict[tuple, bass.Bass] = {}


def _tile_rows(r_pad, k_rows=K):
    """Split r_pad rows into full [128 x k] tiles plus one ragged tail tile."""
    pchunks = r_pad // P
    nt_full = pchunks // k_rows
    k_tail = pchunks % k_rows
    ks = [k_rows] * nt_full + ([k_tail] if k_tail else [])
    return ks


def _build_program(
    r_pad: int,
    repeats: int = 1,
    loop: bool = False,
    gp_tiles: int = 10,
    load_mix=("gpsimd",),
    store_mix=("sync", "scalar"),
    bufs_x: int = 4,
    bufs_w: int = 3,
) -> bass.Bass:
    """One-core Bass program for a shard of r_pad rows (r_pad % 128 == 0).

    `repeats` re-runs the computation body that many times (timing only:
    wall-clock slope over repeats isolates kernel time from the ~100ms flat
    dispatch overhead).  With `loop=True` the repeats use a tc.For_i
    hardware loop (small program; ~2us back-edge barrier per iteration —
    slightly pessimistic vs the unrolled body)."""
    from contextlib import ExitStack

    ks = _tile_rows(r_pad)
    nt = len(ks)
    nc = bacc.Bacc()

    feat = nc.declare_dram_parameter("feat", [r_pad, C], mybir.dt.float32, isOutput=False)
    gamma = nc.declare_dram_parameter("gamma", [1, C], mybir.dt.float32, isOutput=False)
    beta = nc.declare_dram_parameter("beta", [1, C], mybir.dt.float32, isOutput=False)
    out = nc.declare_dram_parameter("out", [r_pad, C], mybir.dt.float16, isOutput=True)

    row0 = [0] * nt
    for t in range(1, nt):
        row0[t] = row0[t - 1] + P * ks[t - 1]

    def feat_view(t):
        r0 = row0[t]
        return feat[r0 : r0 + P * ks[t], :].rearrange("(p k) c -> p (k c)", k=ks[t])

    def out_view(t):
        r0 = row0[t]
        return out[r0 : r0 + P * ks[t], :].rearrange("(p k) c -> p (k c)", k=ks[t])

    with tile.TileContext(nc) as tc, ExitStack() as ctx:
        const = ctx.enter_context(tc.tile_pool(name="const", bufs=1))
        inp = ctx.enter_context(tc.tile_pool(name="inp", bufs=bufs_x))
        resp = ctx.enter_context(tc.tile_pool(name="resp", bufs=1))
        sqp = ctx.enter_context(tc.tile_pool(name="sqp", bufs=2))
        wp = ctx.enter_context(tc.tile_pool(name="wp", bufs=bufs_w))
        psum = ctx.enter_context(tc.tile_pool(name="psum", bufs=1, space="PSUM"))
        small = ctx.enter_context(tc.tile_pool(name="small", bufs=1))

        ones_col = const.tile([P, 1], mybir.dt.bfloat16, name="ones_col", tag="ones_col")
        nc.vector.memset(ones_col, 1.0)
        ones_row = const.tile([1, P], mybir.dt.float32, name="ones_row", tag="ones_row")
        nc.vector.memset(ones_row, 1.0)
        g_row = const.tile([1, C], mybir.dt.float32, name="g_row", tag="g_row")
        nc.sync.dma_start(out=g_row, in_=gamma[:])
        b_row = const.tile([1, C], mybir.dt.float32, name="b_row", tag="b_row")
        nc.sync.dma_start(out=b_row, in_=beta[:])
        # gamma * C, so s = 1 + (gamma*C) * r * (1/sum(r)) needs no /C for the mean
        gC_row = const.tile([1, C], mybir.dt.float32, name="gC_row", tag="gC_row")
        nc.vector.tensor_scalar_mul(gC_row, g_row, float(C))

        # chunks actually written, and the last tile writing each (stop flag)
        nchunks = (max(ks) * C + MM_N - 1) // MM_N
        last_t_for_chunk = [0] * nchunks
        for t in range(nt):
            for j in range((ks[t] * C + MM_N - 1) // MM_N):
                last_t_for_chunk[j] = t

        load_eng = [getattr(nc, e) for e in load_mix]
        store_eng = [getattr(nc, e) for e in store_mix]

        def body():
            # --- pass 1: sum of squares; fp16 copy of every tile stays ---
            acc_all = psum.tile([1, nchunks * MM_N], mybir.dt.float32, name="acc", tag="acc")
            acc = [acc_all[:, j * MM_N : (j + 1) * MM_N] for j in range(nchunks)]
            res_tiles = []
            for t in range(nt):
                f_t = ks[t] * C
                h = resp.tile([P, F], mybir.dt.float16, name="h", tag=f"res{t}")[:, :f_t]
                res_tiles.append(h)
                x = inp.tile([P, F], mybir.dt.float32, name="x", tag="x")[:, :f_t]
                load_eng[t % len(load_eng)].dma_start(out=x, in_=feat_view(t))
                nc.vector.tensor_copy(h, x)
                sq = sqp.tile([P, F], mybir.dt.bfloat16, name="sq", tag="sq")
                nc.scalar.activation(sq[:, :f_t], x, _AFT.Square)
                for j in range((f_t + MM_N - 1) // MM_N):
                    w = min(MM_N, f_t - j * MM_N)
                    nc.tensor.matmul(
                        acc[j][:, :w],
                        lhsT=ones_col[:, :],
                        rhs=sq[:, j * MM_N : j * MM_N + w],
                        start=(t == 0),
                        stop=(t == last_t_for_chunk[j]),
                    )

            # --- combine: [1,64] vector math --------------------------------
            # critical path: reduce -> sqrt(+accum) -> max -> recip -> ts
            #                -> bcast matmul -> psum copy
            sumsq = small.tile([1, C], mybir.dt.float32, name="sumsq", tag="sumsq")
            nc.vector.tensor_reduce(
                out=sumsq,
                in_=acc_all[:, :].rearrange("p (k c) -> p c k", c=C),
                axis=mybir.AxisListType.X,
                op=_ALU.add,
            )
            r_t = small.tile([1, C], mybir.dt.float32, name="r_t", tag="r_t")
            msum = small.tile([1, 1], mybir.dt.float32, name="msum", tag="msum")
            nc.scalar.activation(r_t, sumsq, _AFT.Sqrt, accum_out=msum)
            msafe = small.tile([1, 1], mybir.dt.float32, name="msafe", tag="msafe")
            nc.vector.tensor_scalar_max(msafe, msum, 1e-28)
            minv = small.tile([1, 1], mybir.dt.float32, name="minv", tag="minv")
            nc.vector.reciprocal(minv, msafe)
            t2 = small.tile([1, C], mybir.dt.float32, name="t2", tag="t2")
            nc.vector.tensor_mul(t2, r_t, gC_row)
            sb_cat = small.tile([1, 2 * C], mybir.dt.float32, name="sb_cat", tag="sb_cat")
            nc.vector.tensor_copy(sb_cat[:, C : 2 * C], b_row)
            nc.vector.tensor_scalar(
                sb_cat[:, 0:C], t2, scalar1=minv[:, :], scalar2=1.0,
                op0=_ALU.mult, op1=_ALU.add,
            )
            bc_ps = psum.tile([P, 2 * C], mybir.dt.float32, name="bc_ps", tag="bc_ps")
            nc.tensor.matmul(bc_ps[:, :], lhsT=ones_row[:, :], rhs=sb_cat[:, :], start=True, stop=True)
            sb_bc = small.tile([P, 2 * C], mybir.dt.float16, name="sb_bc", tag="sb_bc")
            nc.scalar.copy(sb_bc, bc_ps)
            s_bc = sb_bc[:, 0:C]
            b_bc = sb_bc[:, C : 2 * C]

            def bcast_ap(col_slice, kk):
                return bass.AP(
                    tensor=col_slice.tensor,
                    offset=col_slice.offset,
                    ap=[col_slice.ap[0], [0, kk], col_slice.ap[1]],
                )

            # --- pass 2: w = h*s + beta in fp16; gp_tiles of nt on GPSIMD ---
            gp_mark = 0
            ns = 0
            for t in range(nt):
                kk = ks[t]
                f_t = kk * C
                h3 = res_tiles[t].rearrange("p (k c) -> p k c", c=C)
                nxt = ((t + 1) * gp_tiles) // nt
                eng = nc.gpsimd if nxt != gp_mark else nc.vector
                gp_mark = nxt
                w = wp.tile([P, F], mybir.dt.float16, name="w", tag="w")[:, :f_t]
                w3 = w.rearrange("p (k c) -> p k c", c=C)
                eng.tensor_tensor(w3, h3, bcast_ap(s_bc, kk), _ALU.mult)
                eng.tensor_tensor(w3, w3, bcast_ap(b_bc, kk), _ALU.add)
                store_eng[ns % len(store_eng)].dma_start(out=out_view(t), in_=w)
                ns += 1

        if loop and repeats > 1:
            with tc.For_i(0, repeats, 1):
                body()
        else:
            for _rep in range(repeats):
                body()

    nc.finalize()
    return nc


def kernel(feat: np.ndarray, offset: np.ndarray, gamma: np.ndarray, beta: np.ndarray) -> np.ndarray:
    feat = np.ascontiguousarray(np.asarray(feat, dtype=np.float32))
    offset = np.asarray(offset)
    gamma = np.ascontiguousarray(np.asarray(gamma, dtype=np.float32)).reshape(1, C)
    beta = np.ascontiguousarray(np.asarray(beta, dtype=np.float32)).reshape(1, C)

    n = feat.shape[0]
    b = offset.shape[0]
    assert b <= N_CORES, f"need <= {N_CORES} segments, got {b}"

    ends = offset.astype(np.int64)
    starts = np.concatenate([[0], ends[:-1]])
    seg_rows = (ends - starts).astype(np.int64)

    r_max = int(seg_rows.max()) if b else P
    r_pad = max(P, ((r_max + P - 1) // P) * P)

    key = (r_pad,)
    nc = _program_cache.get(key)
    if nc is None:
        nc = _build_program(r_pad)
        _program_cache[key] = nc

    in_maps = []
    for i in range(N_CORES):
        shard = np.zeros((r_pad, C), dtype=np.float32)
        if i < b and seg_rows[i] > 0:
            shard[: seg_rows[i]] = feat[starts[i] : ends[i]]
        in_maps.append({"feat": shard, "gamma": gamma, "beta": beta})

    results = run_bass_kernel_spmd(nc, in_maps, core_ids=list(range(N_CORES))).results

    out_full = np.empty((n, C), dtype=np.float32)
    for i in range(b):
        if seg_rows[i] > 0:
            out_full[starts[i] : ends[i]] = results[i]["out"][: seg_rows[i]].astype(np.float32)

    # Rows past offset[-1] (possible with general sorted offsets): the
    # reference's searchsorted yields index b there, which jax clamps to
    # b-1 on gather — those rows are scaled by the last segment's rn but
    # excluded from its sumsq.  Replicate on host.
    tail0 = int(ends[-1]) if b else 0
    if tail0 < n:
        last0, last1 = int(starts[-1]), int(ends[-1])
        sumsq = (feat[last0:last1].astype(np.float64) ** 2).sum(axis=0)
        r = np.sqrt(sumsq)
        rn = (r / (r.mean() + EPS)).astype(np.float32)
        ft = feat[tail0:]
        out_full[tail0:] = ft + gamma * (ft * rn[None, :]) + beta
    return out_full
